# revision 16
# baseline (speedup 1.0000x reference)
"""CFConv (SchNet continuous-filter conv) Trainium2 Bass kernel, 8-core SPMD.

v2 design:
  - Host: bucket edges by (dest 128-node window, src half) per core; fold the
    cosine cutoff C into a host-precomputed one-hot scatter matrix
    oh[e, slot] = C_e * (slot == l_e); build ap_gather int16 indices.
  - Device per core: hT = Win^T-style matmul of xT (bf16) -> SBUF-resident
    transposed node-feature table (fp32, split in two 25088-node halves;
    hi half round-trips via DRAM).  Per (window, half) group: ap_gather
    (GpSimd, ~0.2ns/idx) pulls hgT [f, e] straight from SBUF; filter MLP runs
    in [f, e] layout with Exp/Ln activations using the scale=0.5/bias=0.5
    trick (ln(0.5 e^z + 0.5) == shifted-softplus), so no act-table thrash, no
    bias matmul, no log2 subtraction; m0T = tt * hgT; PE-transpose each
    128-block and scatter via matmul into aggT[f, slot] PSUM; window output
    is aggT @ Wout + ssp, no extra transpose.
  - No cross-core collectives: core c owns output rows [c*6250, (c+1)*6250).
"""

import math
import os
import sys

import numpy as np

sys.path.insert(0, "/opt/trn_rl_repo")

N_ATOMS = 50000
N_EDGES = 1600000
DIM = 128
NF = 128
NG = 50
CUTOFF = 10.0
NCORES = 8
NPC = N_ATOMS // NCORES  # 6250 nodes per core
WIN = 128
NWIN = (NPC + WIN - 1) // WIN  # 49
JHALF = 25088  # 49*512; table half size (int16-indexable)
NPAD = 2 * JHALF  # 50176 padded atoms
SUPER = 512


def _prep(inputs):
    import ml_dtypes

    bf16 = ml_dtypes.bfloat16

    x = np.asarray(inputs["x"], dtype=np.float32)
    r = np.asarray(inputs["r_ij"], dtype=np.float32)
    f = np.asarray(inputs["f_ij"], dtype=np.float32)
    ii = np.asarray(inputs["ind_i"]).astype(np.int64)
    jj = np.asarray(inputs["ind_j"]).astype(np.int64)

    core = ii // NPC
    iloc = ii - core * NPC
    w = iloc // WIN
    l = (iloc % WIN).astype(np.int64)
    half = (jj >= JHALF).astype(np.int64)
    jloc = (jj - half * JHALF).astype(np.int64)
    NG_GROUPS = 2 * NWIN  # 98
    g = half * NWIN + w
    gkey = core * NG_GROUPS + g

    order = np.argsort(gkey, kind="stable")
    counts = np.bincount(gkey, minlength=NCORES * NG_GROUPS).reshape(
        NCORES, NG_GROUPS
    )
    gmax = counts.max(axis=0)
    gpad = np.maximum(128, ((gmax + 127) // 128) * 128)  # [98]
    offs = np.concatenate([[0], np.cumsum(gpad)]).astype(np.int64)
    E_pad = int(offs[-1])
    NBT = E_pad // 128

    sorted_gkey = gkey[order]
    first_idx = np.searchsorted(sorted_gkey, np.arange(NCORES * NG_GROUPS))
    rank = np.arange(N_EDGES) - first_idx[sorted_gkey]
    slot = offs[sorted_gkey % NG_GROUPS] + rank

    C = 0.5 * (np.cos(r * (np.pi / CUTOFF)) + 1.0)
    C = C * (r < CUTOFF)

    per_core = []
    for c in range(NCORES):
        sel = order[core[order] == c]
        sl = slot[core[order] == c]
        ft = np.zeros((50, E_pad), dtype=bf16)
        ft[:, sl] = f[sel].T.astype(bf16)
        # oh[p, gb, s] = C_e * (s == l_e) for edge slot e = gb*128 + p
        oh = np.zeros((128, NBT, 128), dtype=bf16)
        oh[sl % 128, sl // 128, l[sel]] = C[sel].astype(bf16)
        j_pad = np.zeros(E_pad, dtype=np.int16)
        j_pad[sl] = jloc[sel].astype(np.int16)
        jx = np.ascontiguousarray(np.tile(j_pad.reshape(-1, 16).T, (8, 1)))
        per_core.append(
            dict(
                ft=np.ascontiguousarray(ft),
                oh=np.ascontiguousarray(oh.reshape(128, NBT * 128)),
                jx=jx,  # [128, E_pad//16] int16
            )
        )

    xT = np.zeros((DIM, NPAD), dtype=bf16)
    xT[:, :N_ATOMS] = x.T.astype(bf16)
    consts = dict(
        xT=np.ascontiguousarray(xT),
        Wf1=np.asarray(inputs["Wf1"], dtype=np.float32).astype(bf16),
        Wf2=np.asarray(inputs["Wf2"], dtype=np.float32).astype(bf16),
        Win=np.asarray(inputs["Win"], dtype=np.float32).astype(bf16),
        Wout=np.ascontiguousarray(np.asarray(inputs["Wout"], dtype=np.float32)),
        b1=np.asarray(inputs["bf1"], dtype=np.float32).reshape(NF, 1),
        b2=np.asarray(inputs["bf2"], dtype=np.float32).reshape(NF, 1),
        bout=np.asarray(inputs["bout"], dtype=np.float32).reshape(1, NF),
        ones=np.ones((1, 128), dtype=np.float32),
        ident=np.eye(128, dtype=bf16),
    )
    return per_core, consts, gpad, offs, E_pad


def _chunks(gsz):
    out = []
    o = 0
    while o < gsz:
        n = min(SUPER, gsz - o)
        out.append((o, n))
        o += n
    return out


def _restrict_act_tables():
    """Make the act-table placement pass choose natural_log_exp_and_others
    (which holds BOTH Exp and Ln) for every activation, so the Exp/Ln chain
    runs with a single hoisted table load instead of per-op reloads."""
    import concourse.bacc as bacc
    from concourse import hw_specs

    orig = hw_specs.get_activation_tables.__wrapped__

    def restricted(module_arch):
        tabs = orig(module_arch)
        return {
            k: (v if k == "natural_log_exp_and_others" else set())
            for k, v in tabs.items()
        }

    import functools

    cached = functools.cache(restricted)
    bacc.get_activation_tables = cached


def _build(gpad, offs, E_pad, bout_nonzero=False):
    from contextlib import ExitStack

    import concourse.bacc as bacc
    import concourse.bass as bass
    import concourse.mybir as mybir
    import concourse.tile as tile

    _restrict_act_tables()

    dt = mybir.dt
    AF = mybir.ActivationFunctionType
    OP = mybir.AluOpType

    nc = bacc.Bacc()

    ft_d = nc.declare_dram_parameter("ft", [50, E_pad], dt.bfloat16, isOutput=False)
    oh_d = nc.declare_dram_parameter("oh", [128, E_pad], dt.bfloat16, isOutput=False)
    jx_d = nc.declare_dram_parameter(
        "jx", [128, E_pad // 16], dt.int16, isOutput=False
    )
    xT_d = nc.declare_dram_parameter("xT", [DIM, NPAD], dt.bfloat16, isOutput=False)
    Wf1_d = nc.declare_dram_parameter("Wf1", [NG, NF], dt.bfloat16, isOutput=False)
    Wf2_d = nc.declare_dram_parameter("Wf2", [NF, NF], dt.bfloat16, isOutput=False)
    Win_d = nc.declare_dram_parameter("Win", [DIM, NF], dt.bfloat16, isOutput=False)
    Wout_d = nc.declare_dram_parameter("Wout", [NF, NF], dt.float32, isOutput=False)
    b1_d = nc.declare_dram_parameter("b1", [NF, 1], dt.float32, isOutput=False)
    b2_d = nc.declare_dram_parameter("b2", [NF, 1], dt.float32, isOutput=False)
    bout_d = nc.declare_dram_parameter("bout", [1, NF], dt.float32, isOutput=False)
    ones_d = nc.declare_dram_parameter("ones", [1, 128], dt.float32, isOutput=False)
    ident_d = nc.declare_dram_parameter(
        "ident", [128, 128], dt.bfloat16, isOutput=False
    )
    out_d = nc.declare_dram_parameter("out", [NPC, NF], dt.float32, isOutput=True)

    hhi_d = nc.dram_tensor("hhi", [128, JHALF], dt.float32)

    with tile.TileContext(nc) as tc, ExitStack() as ctx:
        cpool = ctx.enter_context(tc.tile_pool(name="consts", bufs=1))
        jpool = ctx.enter_context(tc.tile_pool(name="jx", bufs=1))
        hpool = ctx.enter_context(tc.tile_pool(name="htab", bufs=1))
        ppool = ctx.enter_context(tc.tile_pool(name="partial", bufs=1))
        xpool = ctx.enter_context(tc.tile_pool(name="xload", bufs=2))
        hspool = ctx.enter_context(tc.tile_pool(name="hstage", bufs=1))
        ftpool = ctx.enter_context(tc.tile_pool(name="ft", bufs=3))
        ohpool = ctx.enter_context(tc.tile_pool(name="oh", bufs=3))
        hgpool = ctx.enter_context(tc.tile_pool(name="hg", bufs=2))
        epool = ctx.enter_context(tc.tile_pool(name="e1", bufs=3))
        apool = ctx.enter_context(tc.tile_pool(name="a1", bufs=3))
        e2pool = ctx.enter_context(tc.tile_pool(name="e2", bufs=3))
        ttpool = ctx.enter_context(tc.tile_pool(name="tt", bufs=3))
        mtpool = ctx.enter_context(tc.tile_pool(name="m0T", bufs=3))
        mpool = ctx.enter_context(tc.tile_pool(name="m0", bufs=6))
        aggsp = ctx.enter_context(tc.tile_pool(name="aggs", bufs=2))
        outp = ctx.enter_context(tc.tile_pool(name="outs", bufs=2))
        pz = ctx.enter_context(
            tc.tile_pool(name="pz", bufs=2, space=bass.MemorySpace.PSUM)
        )
        pz2 = ctx.enter_context(
            tc.tile_pool(name="pz2", bufs=2, space=bass.MemorySpace.PSUM)
        )
        ptp = ctx.enter_context(
            tc.tile_pool(name="ptp", bufs=2, space=bass.MemorySpace.PSUM)
        )
        pagg = ctx.enter_context(
            tc.tile_pool(name="pagg", bufs=1, space=bass.MemorySpace.PSUM)
        )
        pout = ctx.enter_context(
            tc.tile_pool(name="pout", bufs=1, space=bass.MemorySpace.PSUM)
        )

        Wf1 = cpool.tile([NG, NF], dt.bfloat16)
        nc.sync.dma_start(Wf1[:], Wf1_d[:])
        Wf2 = cpool.tile([NF, NF], dt.bfloat16)
        nc.sync.dma_start(Wf2[:], Wf2_d[:])
        Win = cpool.tile([DIM, NF], dt.bfloat16)
        nc.sync.dma_start(Win[:], Win_d[:])
        Wout = cpool.tile([NF, NF], dt.float32)
        nc.sync.dma_start(Wout[:], Wout_d[:])
        b1 = cpool.tile([NF, 1], dt.float32)
        nc.sync.dma_start(b1[:], b1_d[:])
        b2 = cpool.tile([NF, 1], dt.float32)
        nc.sync.dma_start(b2[:], b2_d[:])
        bout = cpool.tile([1, NF], dt.float32)
        nc.sync.dma_start(bout[:], bout_d[:])
        ones = cpool.tile([1, 128], dt.float32)
        nc.sync.dma_start(ones[:], ones_d[:])
        ident = cpool.tile([128, 128], dt.bfloat16)
        nc.sync.dma_start(ident[:], ident_d[:])
        half05 = cpool.tile([128, 1], dt.float32)
        nc.gpsimd.memset(half05[:], 0.5)

        lo_len = int(offs[NWIN]) // 16
        hi_len = (E_pad - int(offs[NWIN])) // 16
        jmax = max(lo_len, hi_len)

        partial = ppool.tile([128, NWIN * 128], dt.float32)

        # ---- phase 1: hT = Win^T @ x^T  (bf16 in, fp32 table) ----
        htab_lo = hpool.tile([128, JHALF], dt.float32)
        for nb in range(NPAD // SUPER):  # 98 supertiles of 512 nodes
            xa = xpool.tile([DIM, SUPER], dt.bfloat16)
            nc.sync.dma_start(xa[:], xT_d[:, nb * SUPER : (nb + 1) * SUPER])
            z2f = pz2.tile([128, SUPER], dt.float32)
            nc.tensor.matmul(z2f[:], Win[:], xa[:], start=True, stop=True)
            if nb < NPAD // SUPER // 2:
                nc.vector.tensor_copy(
                    htab_lo[:, nb * SUPER : (nb + 1) * SUPER], z2f[:]
                )
            else:
                hs = hspool.tile([128, SUPER], dt.float32)
                nc.vector.tensor_copy(hs[:], z2f[:])
                o = (nb - NPAD // SUPER // 2) * SUPER
                nc.sync.dma_start(hhi_d[:, o : o + SUPER], hs[:])

        tc.strict_bb_all_engine_barrier()

        def do_half(half, htab):
            hoff = int(offs[NWIN]) * half
            hlen = lo_len if half == 0 else hi_len
            jx = jpool.tile([128, jmax], dt.int16)
            nc.sync.dma_start(
                jx[:, :hlen], jx_d[:, hoff // 16 : hoff // 16 + hlen]
            )

            chunks = []
            for w in range(NWIN):
                g = half * NWIN + w
                gsz = int(gpad[g])
                goff = int(offs[g]) - hoff
                cs = _chunks(gsz)
                for ci, (co, n) in enumerate(cs):
                    chunks.append(
                        dict(
                            w=w,
                            gsz=gsz,
                            goff=goff,
                            co=co,
                            n=n,
                            first=(ci == 0),
                            last=(ci == len(cs) - 1),
                            nblk_w=gsz // 128,
                            ti0=co // 128,
                        )
                    )

            st = [dict() for _ in chunks]
            NCH = len(chunks)
            wagg = {}

            for k in range(NCH + 2):
                # --- stage C (k-2): transpose blocks + copies ---
                if k >= 2:
                    c = chunks[k - 2]
                    s = st[k - 2]
                    s["m0"] = []
                    for b in range(c["n"] // 128):
                        tp = ptp.tile([128, 128], dt.bfloat16)
                        nc.tensor.transpose(
                            tp[:], s["m0T"][:, b * 128 : (b + 1) * 128], ident[:]
                        )
                        m0 = mpool.tile([128, 128], dt.bfloat16)
                        nc.vector.tensor_copy(m0[:], tp[:])
                        s["m0"].append(m0)

                # --- stage A (k): group loads + z1 ---
                if k < NCH:
                    c = chunks[k]
                    s = st[k]
                    if c["first"]:
                        hg = hgpool.tile([128, c["gsz"]], dt.float32, tag="hg")
                        nc.gpsimd.ap_gather(
                            hg[:],
                            htab[:],
                            jx[:, c["goff"] // 16 : (c["goff"] + c["gsz"]) // 16],
                            128,
                            JHALF,
                            1,
                            c["gsz"],
                        )
                        ftg = ftpool.tile([NG, c["gsz"]], dt.bfloat16, tag="ftg")
                        nc.sync.dma_start(
                            ftg[:],
                            ft_d[:, hoff + c["goff"] : hoff + c["goff"] + c["gsz"]],
                        )
                        ohg = ohpool.tile([128, c["gsz"]], dt.bfloat16, tag="ohg")
                        nc.sync.dma_start(
                            ohg[:],
                            oh_d[:, hoff + c["goff"] : hoff + c["goff"] + c["gsz"]],
                        )
                        group_tiles = dict(hg=hg, ftg=ftg, ohg=ohg)
                    s["grp"] = group_tiles
                    n = c["n"]
                    z1f = pz.tile([128, SUPER], dt.float32)
                    s["z1"] = z1f[:, :n]
                    nc.tensor.matmul(
                        s["z1"],
                        Wf1[:],
                        group_tiles["ftg"][:, c["co"] : c["co"] + n],
                        start=True,
                        stop=True,
                    )

                # --- stage C' (k-2): scatter matmuls + window end ---
                if k >= 2:
                    c = chunks[k - 2]
                    s = st[k - 2]
                    w = c["w"]
                    if c["first"]:
                        wagg[w] = pagg.tile([128, 128], dt.float32, name="aggT", tag="aggT")
                    aggT = wagg[w]
                    ohg = s["grp"]["ohg"]
                    for b in range(c["n"] // 128):
                        ti = c["ti0"] + b
                        nc.tensor.matmul(
                            aggT[:],
                            s["m0"][b][:],
                            ohg[:, c["co"] + b * 128 : c["co"] + (b + 1) * 128],
                            start=(ti == 0),
                            stop=(ti == c["nblk_w"] - 1),
                        )

                # --- stage B1 (k): e1, a1 ---
                if k < NCH:
                    s = st[k]
                    n = chunks[k]["n"]
                    e1f = epool.tile([128, SUPER], dt.bfloat16)
                    e1 = e1f[:, :n]
                    nc.scalar.activation(e1, s["z1"], AF.Exp, bias=b1[:, 0:1])
                    a1f = apool.tile([128, SUPER], dt.bfloat16)
                    s["a1"] = a1f[:, :n]
                    nc.scalar.activation(
                        s["a1"], e1, AF.Ln, bias=half05[:, 0:1], scale=0.5
                    )

                # --- stage B2 (k-1): z2 ---
                if 1 <= k <= NCH:
                    s = st[k - 1]
                    n = chunks[k - 1]["n"]
                    z2f = pz2.tile([128, SUPER], dt.float32)
                    s["z2"] = z2f[:, :n]
                    nc.tensor.matmul(
                        s["z2"], Wf2[:], s["a1"], start=True, stop=True
                    )

                # --- window end for (k-2) ---
                if k >= 2:
                    c = chunks[k - 2]
                    if c["last"]:
                        w = c["w"]
                        aggT = wagg.pop(w)
                        if half == 0:
                            nc.vector.tensor_copy(
                                partial[:, w * 128 : (w + 1) * 128], aggT[:]
                            )
                        else:
                            aggs = aggsp.tile([128, 128], dt.float32)
                            nc.vector.tensor_tensor(
                                aggs[:],
                                aggT[:],
                                partial[:, w * 128 : (w + 1) * 128],
                                OP.add,
                            )
                            opp = pout.tile([128, 128], dt.float32)
                            if bout_nonzero:
                                nc.tensor.matmul(
                                    opp[:], ones[:], bout[:], start=True, stop=False
                                )
                                nc.tensor.matmul(
                                    opp[:], aggs[:], Wout[:], start=False, stop=True
                                )
                            else:
                                nc.tensor.matmul(
                                    opp[:], aggs[:], Wout[:], start=True, stop=True
                                )
                            eo = outp.tile([128, 128], dt.float32)
                            nc.scalar.activation(eo[:], opp[:], AF.Exp)
                            oo = outp.tile([128, 128], dt.float32, tag="oo")
                            nc.scalar.activation(
                                oo[:], eo[:], AF.Ln, bias=half05[:, 0:1], scale=0.5
                            )
                            nrows = min(WIN, NPC - w * WIN)
                            nc.sync.dma_start(
                                out_d[w * WIN : w * WIN + nrows, :], oo[:nrows, :]
                            )

                # --- stage B3 (k-1): e2, tt, m0T ---
                if 1 <= k <= NCH:
                    c = chunks[k - 1]
                    s = st[k - 1]
                    n = c["n"]
                    e2f = e2pool.tile([128, SUPER], dt.bfloat16)
                    e2 = e2f[:, :n]
                    nc.scalar.activation(e2, s["z2"], AF.Exp, bias=b2[:, 0:1])
                    ttf = ttpool.tile([128, SUPER], dt.bfloat16)
                    tt = ttf[:, :n]
                    nc.scalar.activation(
                        tt, e2, AF.Ln, bias=half05[:, 0:1], scale=0.5
                    )
                    m0Tf = mtpool.tile([128, SUPER], dt.bfloat16)
                    s["m0T"] = m0Tf[:, :n]
                    hg = s["grp"]["hg"]
                    nc.vector.tensor_tensor(
                        s["m0T"], tt, hg[:, c["co"] : c["co"] + n], OP.mult
                    )
                    s.pop("a1", None)
                    s.pop("z1", None)
                    s.pop("z2", None)

        do_half(0, htab_lo)
        nc.sync.dma_start(htab_lo[:], hhi_d[:])
        do_half(1, htab_lo)

    if not nc.is_finalized():
        nc.finalize()
    return nc


def kernel(**inputs):
    from concourse.bass_utils import run_bass_kernel_spmd

    per_core, consts, gpad, offs, E_pad = _prep(inputs)
    bout_nonzero = bool(np.any(consts["bout"]))

    nc = _build(gpad, offs, E_pad, bout_nonzero=bout_nonzero)

    in_maps = []
    for c in range(NCORES):
        m = dict(per_core[c])
        m.update(consts)
        in_maps.append(m)

    trace = os.environ.get("CFCONV_TRACE", "0") == "1"
    res = run_bass_kernel_spmd(nc, in_maps, list(range(NCORES)), trace=trace)
    if trace and res.exec_time_ns is not None:
        print(f"HW exec time: {res.exec_time_ns} ns")
        kernel.last_exec_time_ns = res.exec_time_ns
    kernel.last_results = res
    out = np.concatenate(
        [np.asarray(res.results[c]["out"]) for c in range(NCORES)], axis=0
    )
    return out.astype(np.float32)
